# revision 17
# baseline (speedup 1.0000x reference)
"""Bahdanau-attention score kernel (softmax(v . tanh(W[h;enc]+b))) for 8 TRN2 cores.

v10: bf16 GEMM inputs (halves HBM traffic: enc 19.8MB->9.9MB/core, W
2.2->1.1MB; empirically 9.0e-3 max rel err vs the 2e-2 gate, dominated
by input quantization -- hb stays f32 on host, one-hot rows exact in
bf16), PE warm-up matmuls during the DMA prime window so the HAM clock
gate promotes to 2.4GHz before tile 0 (v9 lost ~10us to a 1.2GHz cold
window after an 8.8us DMA stall demoted it), 3-deep enc tile pool for
true 2-batch prefetch without burst stalls, per-batch softmax phase-1
(16 score cols each, emitted mid-next-batch so the in-order PE queue
never waits) instead of v9's two 32-col halves (the second of which
serialized the entire tail), and the v9 staples: host pre-transposed
enc, host-precomputed h_proj+b folded in via one-hot contraction rows,
DVE mul with the free-axis reduce alternating DVE/ACT, block-ones
matmul softmax denominator, bank-major emission on the final tile.

Self-contained: hardcodes shapes B=32, S=2048, ENC2=600, DD=900.
Sharding: data-parallel over batch (4 batches/core), weights replicated.
"""

import numpy as np
import ml_dtypes

import concourse.bass as bass  # noqa: F401
import concourse.mybir as mybir
import concourse.tile as tile
from concourse import bacc
from concourse.bass_utils import run_bass_kernel_spmd

F32 = mybir.dt.float32
F32R = mybir.dt.float32r
BF16 = mybir.dt.bfloat16
NP_BF16 = ml_dtypes.bfloat16
AF = mybir.ActivationFunctionType
ALU = mybir.AluOpType
AX = mybir.AxisListType

NCORES = 8
B, S, E2, DD = 32, 2048, 600, 900
IN_DIM = DD + E2            # 1500
BL = B // NCORES            # 4 batches per core
SROWS = BL * S              # 8192 s-rows per core
P = 128
NT = S // P                 # 16 s-tiles per batch
NCOL = SROWS // P           # 64 score columns
KA = 92                     # chunk-4 contraction: 88 e-rows + 4 one-hot rows
NSP = [(0, 512), (512, 388)]  # N splits of 900 (PSUM bank = 512 f32)
NWARM = 10                  # HAM warm-up matmuls bridging the DMA prime window


def build():
    nc = bacc.Bacc("TRN2", target_bir_lowering=False)
    # packed layouts: DMA rows are adjacent in DRAM (row stride == row
    # length) so the HWDGE reads sequential pages instead of 16KB strides
    KR = 4 * P + KA             # 604 packed rows per batch (4x128 enc + tail)
    e0a_ext = nc.dram_tensor("e0a", [KR, S // 2], BF16, kind="ExternalInput")
    e0b_ext = nc.dram_tensor("e0b", [KR, S // 2], BF16, kind="ExternalInput")
    encr_ext = nc.dram_tensor("encr", [(BL - 1) * KR, S], BF16,
                              kind="ExternalInput")
    rhsm_ext = nc.dram_tensor("rhsm", [512, DD], BF16, kind="ExternalInput")
    rhs4_ext = nc.dram_tensor("rhs4", [KA, DD], BF16, kind="ExternalInput")
    v_ext = nc.dram_tensor("v", [1, DD], F32R, kind="ExternalInput")
    ones_ext = nc.dram_tensor("ones", [1, P], F32R, kind="ExternalInput")
    bones_ext = nc.dram_tensor("bones", [NT, NT], F32, kind="ExternalInput")
    ident_ext = nc.dram_tensor("ident", [P, P], F32, kind="ExternalInput")
    out_ext = nc.dram_tensor("out", [BL, S], F32, kind="ExternalOutput")

    with tile.TileContext(nc) as tc:
        with (
            tc.tile_pool(name="stat", bufs=1) as stat,
            tc.tile_pool(name="encp", bufs=3) as encp,
            tc.tile_pool(name="zp", bufs=4) as zp,
            tc.tile_pool(name="jp", bufs=3) as jp,
            tc.tile_pool(name="ps_e", bufs=4, space="PSUM") as ps_e,
        ):
            # tiny v/ones first (feed the PE warm-up block), then the
            # critical stream: rhs chunk c interleaved with batch-0 enc
            # chunk c (halves, so tile 0 gates on 5 half-tile DMAs not
            # 5 full ones).
            v_row = stat.tile([1, DD], F32R)
            nc.sync.dma_start(out=v_row[:, :], in_=v_ext.ap())
            ones_t = stat.tile([1, P], F32R)
            nc.sync.dma_start(out=ones_t[:, :], in_=ones_ext.ap())

            rhs = []
            cm_tiles = {}
            bones = stat.tile([NT, NT], F32)
            ident_f = stat.tile([P, P], F32)
            # Three parallel dispatch queues (Sync/Scalar/GpSimd HWDGE,
            # ~100-120GB/s each in the head) with the tile-0 critical
            # 2.5MB balanced across them in consumption order: chunk c's
            # weights + batch-0 first half land c-major so the PE can
            # accumulate as data arrives, second halves follow.
            for c in range(5):
                kp = P if c < 4 else KA
                rhs.append(stat.tile([kp, DD], BF16, name=f"rhs{c}"))
            b0 = [encp.tile([P if c < 4 else KA, S], BF16, tag=f"cm{c}",
                            name=f"cm{c}_0") for c in range(5)]

            def rhs_src(c):
                return (rhsm_ext.ap()[c * P:(c + 1) * P, :] if c < 4
                        else rhs4_ext.ap())

            def crows(c):
                return (c * P, (c + 1) * P) if c < 4 else (4 * P, KR)

            def cm_src(c, b):
                r0, r1 = crows(c)
                return encr_ext.ap()[(b - 1) * KR + r0:(b - 1) * KR + r1, :]

            H = 8 * P
            # Single HWDGE engine serves all dispatch queues round-robin,
            # so parallel queues add no bandwidth -- one priority-ordered
            # Sync stream wins: chunk c's weights + batch-0 first half
            # land c-major (tiles 0-7 gate only on first halves), second
            # halves (tiles 8-15) follow.
            for c in range(5):
                r0, r1 = crows(c)
                nc.sync.dma_start(out=rhs[c][:, :], in_=rhs_src(c))
                nc.sync.dma_start(out=b0[c][:, 0:H], in_=e0a_ext.ap()[r0:r1, :])
            for c in range(5):
                r0, r1 = crows(c)
                nc.sync.dma_start(out=b0[c][:, H:S], in_=e0b_ext.ap()[r0:r1, :])
            nc.sync.dma_start(out=bones[:, :], in_=bones_ext.ap())
            nc.sync.dma_start(out=ident_f[:, :], in_=ident_ext.ap())
            cm_tiles[0] = b0

            # v_rep = ones^T @ v, then NWARM warm-up matmuls chained as
            # one accumulation group into a scratch PSUM slot (chained so
            # DCE can't drop them; read once at the end). The warm-up
            # lhsT is a full-K memset tile: the HAM activity monitor
            # tracks PE array row activity, so K=1 matmuls never promote
            # the clock gate -- K=128 ones do, before tile 0 arrives.
            wsrc = stat.tile([P, 5 * P], BF16, name="wsrc")
            nc.gpsimd.memset(wsrc[:, :], 0.0)
            v_rep = stat.tile([P, DD], F32)
            psv = ps_e.tile([P, DD], F32, tag="ep", name="ps_vrep")
            for (no, nn) in NSP:
                nc.tensor.matmul(psv[:, no:no + nn], ones_t[0:1, :],
                                 v_row[0:1, no:no + nn],
                                 start=True, stop=True)
            nc.scalar.copy(v_rep[:, :], psv[:, :])
            psw = ps_e.tile([P, DD], F32, tag="ep", name="ps_warm")
            for i in range(NWARM):
                nc.tensor.matmul(psw[:, 0:512], wsrc[:, 0:P],
                                 wsrc[:, P:5 * P],
                                 start=(i == 0), stop=(i == NWARM - 1))

            def issue_batch(b):
                tiles = []
                for c in range(5):
                    kp = P if c < 4 else KA
                    t_ = encp.tile([kp, S], BF16, tag=f"cm{c}", name=f"cm{c}_{b}")
                    nc.sync.dma_start(out=t_[:, :], in_=cm_src(c, b))
                    tiles.append(t_)
                cm_tiles[b] = tiles

            issue_batch(1)

            scores = [stat.tile([P, NT], F32, name=f"scores{h}")
                      for h in range(BL)]
            sc_ab = stat.tile([P, 2], F32)
            e1 = [stat.tile([NT, P], F32, name=f"e1_{h}") for h in range(BL)]
            rs = [stat.tile([NT, 1], F32, name=f"rs{h}") for h in range(BL)]
            rfac = [stat.tile([NT, 1], F32, name=f"rfac{h}") for h in range(BL)]
            outf = [stat.tile([NT, P], F32, name=f"outf{h}") for h in range(BL)]
            dve_scr = stat.tile([1, 4], F32)
            qwake = stat.tile([1, 1], F32R)

            # engine primes: absorb DMA sems before the hot loop; the
            # psw read also keeps the warm-up chain live through DCE
            nc.vector.tensor_copy(out=dve_scr[0:1, 0:1], in_=v_rep[0:1, 0:1])
            nc.vector.tensor_copy(out=dve_scr[0:1, 1:2], in_=bones[0:1, 0:1])
            nc.vector.tensor_copy(out=dve_scr[0:1, 2:3], in_=psw[0:1, 0:1])

            # ---------------- main loop ----------------
            def softmax_a(h):
                # transpose + exp one batch's 16 scores columns; the
                # per-batch scores tile was last written a full batch ago
                # so the PE transpose never waits
                pss = ps_e.tile([P, DD], F32, tag="ep", name=f"ps_sm{h}")
                nc.tensor.transpose(pss[0:NT, 0:P],
                                    scores[h][:, :],
                                    ident_f[:, :])
                nc.scalar.activation(
                    e1[h][:, :], pss[0:NT, 0:P], AF.Exp,
                    accum_out=rs[h][:, :],
                )

            def softmax_b(h):
                # denominator + scale + out DMA; emitted several tiles
                # after softmax_a so the psd matmul finds rs ready and
                # never blocks the in-order PE queue
                # bones is all-ones: psd = sum(rs[h]) replicated over 16
                # partitions
                psd = ps_e.tile([P, DD], F32, tag="ep", name=f"ps_bs{h}")
                nc.tensor.matmul(psd[0:NT, 0:1], bones[0:NT, 0:NT],
                                 rs[h][:, :], start=True, stop=True)
                nc.vector.reciprocal(rfac[h][:, :], psd[0:NT, 0:1])
                nc.vector.tensor_scalar_mul(outf[h][:, :], e1[h][:, :],
                                            rfac[h][:, 0:1])
                nc.sync.dma_start(
                    out=out_ext.ap()[h:h + 1, :].rearrange(
                        "b (t p) -> (b t) p", p=P),
                    in_=outf[h][:, :],
                )

            for b in range(BL):
                if b + 2 < BL:
                    issue_batch(b + 2)
                cm = cm_tiles.pop(b)
                for t in range(NT):
                    k = NT * b + t
                    last = k == NCOL - 1
                    eps = ps_e.tile([P, DD], F32, tag="ep")
                    if last:
                        # bank-major emission: the (0,512) accumulation
                        # group finishes 5 matmuls early, so its tanh/mul/
                        # reduce overlap the (512,388) group's streams --
                        # shortens the serial end-of-kernel chain
                        for (no, nn) in NSP:
                            for c in range(5):
                                kp = P if c < 4 else KA
                                nc.tensor.matmul(
                                    eps[:, no:no + nn],
                                    cm[c][0:kp, t * P:(t + 1) * P],
                                    rhs[c][:, no:no + nn],
                                    start=(c == 0), stop=(c == 4),
                                )
                        # tail chain: bank A (done 5 matmuls early)
                        # reduces via ACT accum-copy, bank B on DVE, so
                        # the two halves drain on parallel engines
                        z = zp.tile([P, DD], F32, tag="z")
                        junk = jp.tile([P, DD], F32, tag="junk")
                        dump = jp.tile([P, DD], F32, tag="dump")
                        (no0, nn0), (no1, nn1) = NSP
                        nc.scalar.activation(z[:, no0:no0 + nn0],
                                             eps[:, no0:no0 + nn0], AF.Tanh)
                        nc.vector.tensor_mul(junk[:, no0:no0 + nn0],
                                             z[:, no0:no0 + nn0],
                                             v_rep[:, no0:no0 + nn0])
                        nc.scalar.activation(z[:, no1:no1 + nn1],
                                             eps[:, no1:no1 + nn1], AF.Tanh)
                        nc.scalar.activation(
                            dump[:, no0:no0 + nn0], junk[:, no0:no0 + nn0],
                            AF.Copy, accum_out=sc_ab[:, 0:1],
                        )
                        nc.vector.tensor_mul(junk[:, no1:no1 + nn1],
                                             z[:, no1:no1 + nn1],
                                             v_rep[:, no1:no1 + nn1])
                        nc.vector.tensor_reduce(
                            out=sc_ab[:, 1:2], in_=junk[:, no1:no1 + nn1],
                            axis=AX.X, op=ALU.add,
                        )
                        nc.vector.tensor_reduce(
                            out=scores[b][:, t:t + 1], in_=sc_ab[:, :],
                            axis=AX.X, op=ALU.add,
                        )
                        continue
                    for c in range(5):
                        kp = P if c < 4 else KA
                        lhs = cm[c][0:kp, t * P:(t + 1) * P]
                        for (no, nn) in NSP:
                            nc.tensor.matmul(
                                eps[:, no:no + nn],
                                lhs,
                                rhs[c][:, no:no + nn],
                                start=(c == 0), stop=(c == 4),
                            )
                    z = zp.tile([P, DD], F32, tag="z")
                    nc.scalar.activation(z[:, :], eps[:, :], AF.Tanh)
                    junk = jp.tile([P, DD], F32, tag="junk")
                    nc.vector.tensor_mul(junk[:, :], z[:, :], v_rep[:, :])
                    # batch BL-1 flips parity so tile 62's reduce lands
                    # on ACT, leaving DVE clear for the last tile's chain
                    act_red = (t % 2 == 1) if b < BL - 1 else (t % 2 == 0)
                    if act_red:
                        dump = jp.tile([P, DD], F32, tag="dump")
                        nc.scalar.activation(
                            dump[:, :], junk[:, :], AF.Copy,
                            accum_out=scores[b][:, t:t + 1],
                        )
                    else:
                        nc.vector.tensor_reduce(
                            out=scores[b][:, t:t + 1], in_=junk[:, :],
                            axis=AX.X, op=ALU.add,
                        )
                    # previous batch's 16 cols are long reduced by tile 6
                    # -> the PE transpose never stalls the queue
                    if b >= 1 and t == 6:
                        softmax_a(b - 1)
                    if b >= 1 and t == 12:
                        softmax_b(b - 1)
                    if b == BL - 1 and t == 13:
                        # wake the idle Sync DMA queue so the final 8KB
                        # out DMA doesn't pay ~1.5us cold-queue latency
                        nc.sync.dma_start(out=qwake[0:1, 0:1],
                                          in_=ones_ext.ap()[0:1, 0:1])

            softmax_a(BL - 1)
            softmax_b(BL - 1)
    return nc


_CACHE = {}


def _get_nc():
    if "nc" not in _CACHE:
        nc = build()
        nc.compile()
        _CACHE["nc"] = nc
    return _CACHE["nc"]


def make_in_maps(hidden, encoder_outputs, attn_W, attn_b, v):
    hidden = np.asarray(hidden, dtype=np.float32)
    attn_W = np.asarray(attn_W, dtype=np.float32)
    attn_b = np.asarray(attn_b, dtype=np.float32)
    v = np.asarray(v, dtype=np.float32).reshape(1, DD)
    enc = np.asarray(encoder_outputs, dtype=np.float32)

    WT = np.ascontiguousarray(attn_W.T)          # [1500, 900]
    rhsm = WT[DD:DD + 512].astype(NP_BF16)       # We^T rows 0:512
    we_tail = WT[DD + 512:IN_DIM]                # [88, 900] f32
    hb_all = hidden @ attn_W[:, :DD].T + attn_b  # [32, 900] f32 (exact)

    bones = np.ones((NT, NT), dtype=np.float32)

    KR = 4 * 128 + KA  # 604
    in_maps = []
    for cidx in range(NCORES):
        bs = slice(cidx * BL, (cidx + 1) * BL)
        encT = enc[bs].reshape(SROWS, E2).T.astype(NP_BF16)  # [600, 8192]
        # packed per-batch blocks [604, 2048]: 512 enc rows, 88 tail rows,
        # 4 one-hot rows (row 600+bb is all-ones for batch bb)
        blocks = []
        for bb in range(BL):
            blk = np.zeros((KR, S), dtype=NP_BF16)
            blk[:600] = encT[:, bb * S:(bb + 1) * S]
            blk[600 + bb] = 1.0
            blocks.append(blk)
        rhs4 = np.concatenate([we_tail, hb_all[bs]], axis=0)  # [92, 900]
        in_maps.append({
            "e0a": np.ascontiguousarray(blocks[0][:, :S // 2]),
            "e0b": np.ascontiguousarray(blocks[0][:, S // 2:]),
            "encr": np.concatenate(blocks[1:], axis=0),
            "rhsm": np.ascontiguousarray(rhsm),
            "rhs4": np.ascontiguousarray(rhs4.astype(NP_BF16)),
            "v": v,
            "ones": np.ones((1, P), dtype=np.float32),
            "bones": bones,
            "ident": np.eye(P, dtype=np.float32),
        })
    return in_maps


def run(in_maps, trace=False, **kw):
    nc = _get_nc()
    return run_bass_kernel_spmd(nc, in_maps, core_ids=list(range(NCORES)),
                                trace=trace, **kw)


def kernel(hidden, encoder_outputs, attn_W, attn_b, v):
    in_maps = make_in_maps(hidden, encoder_outputs, attn_W, attn_b, v)
    try:
        res = run(in_maps)
    except Exception:
        # transient device states (e.g. a previously wedged core) sometimes
        # clear on retry
        res = run(in_maps)
    out = np.concatenate([res.results[c]["out"] for c in range(NCORES)], axis=0)
    return np.ascontiguousarray(out, dtype=np.float32)
